# revision 19
# baseline (speedup 1.0000x reference)
"""Causal MHA (B=2, S=2048, D=1024, H=16) sharded over 8 NeuronCores.

Sharding: head-parallel. Core c owns heads {2c, 2c+1} for both batches:
Wq/Wk/Wv split by head rows (column-parallel), Wo split by head columns
(row-parallel); per-core fp16 partial outputs are summed on the host.

Per-core pipeline (matmul inputs fp16, PSUM accumulation f32):
  xT[d, s] <-- dma-transpose straight from DRAM (x pre-cast fp16 on host)
  QT/KT/VT[dkh, s] = W^T-stationary matmuls over xT
  RoPE on QT/KT via host cos/sin tables + pair-swap matmul (P2)
  VT -> DRAM scratch (with a ones row per head) -> dma-transpose back as
  V' [s, 65] per (seq-tile, head); col 64 = ones = softmax denominator
  S^T[kr, qr] = KT-stationary @ QT   (scores transposed, per head)
  P^T = exp(S^T/8)  (no max subtraction; logits are O(1)) + causal mask
  attn^T[65, qr] += V'.T @ P^T
  PE-transpose attn^T -> [qr, 65]; divide by col 64; restack both heads;
  PE-transpose back -> attnN^T[dkh, qr]; out[qr, o] = attnN^T.T @ WoT
"""

import sys

import numpy as np

sys.path.insert(0, "/opt/trn_rl_repo")

B, S, D, H = 2, 2048, 1024, 16
DK = D // H            # 64
NCORES = 8
HPC = H // NCORES      # 2 heads per core
DKH = HPC * DK         # 128 local head-dim
THETA = 10000.0
SCALE = 1.0 / float(np.sqrt(DK))

NT = S // 128          # 16 seq tiles of 128
NCH = S // 1024        # 2 qr chunks of 1024
VROW = 80              # per-head row block in the V DRAM scratch (16-aligned)


def _rope_tables():
    pos = np.arange(S, dtype=np.float64)
    dim = np.arange(0, DK, 2, dtype=np.float64)
    inv_freq = 1.0 / THETA ** (dim / DK)
    angle = pos[None, :] * inv_freq[:, None]        # [DK/2, S]
    angle = np.repeat(angle, 2, axis=0)             # [DK, S] interleaved rows
    cos1, sin1 = np.cos(angle), np.sin(angle)
    cosT = np.concatenate([cos1, cos1], axis=0).astype(np.float16)  # [128, S]
    sinT = np.concatenate([sin1, sin1], axis=0).astype(np.float16)
    return cosT, sinT


def _p2t():
    # pair rotation: out[2i] = -in[2i+1]; out[2i+1] = +in[2i], per 64-row head.
    # matmul computes lhsT.T @ rhs, so pass P2^T.
    p = np.zeros((DK, DK), dtype=np.float32)
    for i in range(DK // 2):
        p[2 * i, 2 * i + 1] = -1.0
        p[2 * i + 1, 2 * i] = 1.0
    p2 = np.zeros((DKH, DKH), dtype=np.float32)
    p2[:DK, :DK] = p
    p2[DK:, DK:] = p
    return np.ascontiguousarray(p2.T).astype(np.float16)


def _col_pieces(qs):
    """Split [qs, 1024) into <=512-wide matmul column pieces."""
    if qs >= 512:
        return [(qs, 1024)]
    return [(qs, 512), (512, 1024)]


def _build_nc():
    from contextlib import ExitStack

    import concourse.tile as tile
    from concourse import bacc, mybir
    from concourse.masks import make_identity

    fp16 = mybir.dt.float16
    f32 = mybir.dt.float32
    EXP = mybir.ActivationFunctionType.Exp
    IS_GE = mybir.AluOpType.is_ge

    nc = bacc.Bacc(
        "TRN2", target_bir_lowering=False, debug=False, num_devices=NCORES
    )
    x_d = nc.dram_tensor("x", [B, S, D], fp16, kind="ExternalInput")
    wqt_d = nc.dram_tensor("wqt", [D, DKH], fp16, kind="ExternalInput")
    wkt_d = nc.dram_tensor("wkt", [D, DKH], fp16, kind="ExternalInput")
    wvt_d = nc.dram_tensor("wvt", [D, DKH], fp16, kind="ExternalInput")
    wot_d = nc.dram_tensor("wot", [DKH, D], fp16, kind="ExternalInput")
    cos_d = nc.dram_tensor("cosT", [DKH, S], fp16, kind="ExternalInput")
    sin_d = nc.dram_tensor("sinT", [DKH, S], fp16, kind="ExternalInput")
    p2t_d = nc.dram_tensor("p2t", [DKH, DKH], fp16, kind="ExternalInput")
    out_d = nc.dram_tensor("out", [B, S, D], fp16, kind="ExternalOutput")

    with tile.TileContext(nc) as tc, ExitStack() as ctx:
        consts = ctx.enter_context(tc.tile_pool(name="consts", bufs=1))
        xtp = ctx.enter_context(tc.tile_pool(name="xt", bufs=8))
        qkraw = ctx.enter_context(tc.tile_pool(name="qkraw", bufs=2))
        qkrope = ctx.enter_context(tc.tile_pool(name="qkrope", bufs=4))
        vtp = ctx.enter_context(tc.tile_pool(name="vt", bufs=2))
        vnp = ctx.enter_context(tc.tile_pool(name="vn", bufs=4))
        ptp = ctx.enter_context(tc.tile_pool(name="pt", bufs=4))
        ropetmp = ctx.enter_context(tc.tile_pool(name="ropetmp", bufs=3))
        epi = ctx.enter_context(tc.tile_pool(name="epi", bufs=4))
        stackp = ctx.enter_context(tc.tile_pool(name="stack", bufs=18))
        outp = ctx.enter_context(tc.tile_pool(name="outsb", bufs=3))
        dramp = ctx.enter_context(tc.tile_pool(name="vdram", bufs=2, space="DRAM"))

        ps_sc = ctx.enter_context(tc.tile_pool(name="ps_sc", bufs=2, space="PSUM"))
        ps_at = ctx.enter_context(tc.tile_pool(name="ps_at", bufs=1, space="PSUM"))
        ps_aux = ctx.enter_context(tc.tile_pool(name="ps_aux", bufs=2, space="PSUM"))

        # ---- constants ----
        wq_sb = consts.tile([128, 8, DKH], fp16, tag="wq")
        wk_sb = consts.tile([128, 8, DKH], fp16, tag="wk")
        wv_sb = consts.tile([128, 8, DKH], fp16, tag="wv")
        nc.gpsimd.dma_start(wq_sb, wqt_d[:, :].rearrange("(j p) m -> p j m", p=128))
        nc.gpsimd.dma_start(wk_sb, wkt_d[:, :].rearrange("(j p) m -> p j m", p=128))
        nc.gpsimd.dma_start(wv_sb, wvt_d[:, :].rearrange("(j p) m -> p j m", p=128))
        wo_sb = consts.tile([DKH, D], fp16, tag="wo")
        nc.gpsimd.dma_start(wo_sb, wot_d[:, :])
        cos_sb = consts.tile([DKH, S], fp16, tag="cos")
        sin_sb = consts.tile([DKH, S], fp16, tag="sin")
        nc.gpsimd.dma_start(cos_sb, cos_d[:, :])
        nc.gpsimd.dma_start(sin_sb, sin_d[:, :])
        p2_sb = consts.tile([DKH, DKH], fp16, tag="p2")
        nc.gpsimd.dma_start(p2_sb, p2t_d[:, :])
        ident = consts.tile([128, 128], fp16, tag="ident")
        make_identity(nc, ident)
        ones_sb = consts.tile([1, S], fp16, tag="ones")
        nc.vector.memset(ones_sb, 1.0)

        for b in range(B):
            # ================= phase A: projections =================
            # xT chunks straight from DRAM: xt[c][p, j, r] = x[b, 512c+r, 128j+p]
            xts = []
            for cch in range(4):
                xt = xtp.tile([128, 8, 512], fp16, tag="xt")
                nc.sync.dma_start_transpose(xt, x_d[b, 512 * cch : 512 * (cch + 1), :])
                xts.append(xt)

            def project(w_sb, dst):
                for cch in range(4):
                    ps = ps_aux.tile([128, 512], f32, tag="aux")
                    for j in range(8):
                        nc.tensor.matmul(
                            ps,
                            w_sb[:, j, :],
                            xts[cch][:, j, :],
                            start=(j == 0),
                            stop=(j == 7),
                        )
                    nc.vector.tensor_copy(dst[:, 512 * cch : 512 * (cch + 1)], ps)

            qt_raw = qkraw.tile([DKH, S], fp16, tag="qkraw")
            project(wq_sb, qt_raw)
            kt_raw = qkraw.tile([DKH, S], fp16, tag="qkraw")
            project(wk_sb, kt_raw)
            vt = vtp.tile([DKH, S], fp16, tag="vt")
            project(wv_sb, vt)

            def rope(src, dst):
                for cch in range(4):
                    sl = slice(512 * cch, 512 * (cch + 1))
                    ps = ps_aux.tile([128, 512], f32, tag="aux")
                    nc.tensor.matmul(ps, p2_sb, src[:, sl], start=True, stop=True)
                    t1 = ropetmp.tile([DKH, 512], fp16, tag="ropetmp")
                    nc.vector.tensor_mul(t1, src[:, sl], cos_sb[:, sl])
                    t2 = ropetmp.tile([DKH, 512], fp16, tag="ropetmp")
                    nc.vector.tensor_mul(t2, ps, sin_sb[:, sl])
                    nc.vector.tensor_add(dst[:, sl], t1, t2)

            qtr = qkrope.tile([DKH, S], fp16, tag="qkrope")
            rope(qt_raw, qtr)
            ktr = qkrope.tile([DKH, S], fp16, tag="qkrope")
            rope(kt_raw, ktr)

            # V roundtrip through DRAM: rows [80h, 80h+64) = head h V^T,
            # row 80h+64 = ones (softmax denominator), rows +65.. unused.
            vdr = dramp.tile([2 * VROW, S], fp16, tag="vdr")
            nc.gpsimd.dma_start(vdr[0:64, :], vt[0:64, :])
            nc.gpsimd.dma_start(vdr[64:65, :], ones_sb)
            nc.gpsimd.dma_start(vdr[VROW : VROW + 64, :], vt[64:128, :])
            nc.gpsimd.dma_start(vdr[VROW + 64 : VROW + 65, :], ones_sb)
            vns = []
            for h in range(HPC):
                vn = vnp.tile([128, NT, VROW], fp16, tag="vn")
                nc.sync.dma_start_transpose(
                    vn, vdr[VROW * h : VROW * (h + 1), :]
                )
                vns.append(vn)

            # ================= phase B: attention =================
            stacks = {}
            for h in range(HPC):
                hsl = slice(DK * h, DK * (h + 1))
                for cch in range(NCH):  # qr chunks of 1024
                    qbase = 1024 * cch
                    at_ps = ps_at.tile([128, 1024], f32, tag="at")
                    n_kt = min(NT, 8 * (cch + 1))

                    def emit_pv(t, pt, qs):
                        for lo, hi in _col_pieces(qs):
                            nc.tensor.matmul(
                                at_ps[0:65, lo:hi],
                                vns[h][:, t, 0:65],
                                pt[:, lo:hi],
                                start=(t == 0),
                                stop=(t == n_kt - 1),
                            )

                    pending = None  # software-pipeline PV one kr-tile back
                    for t in range(n_kt):
                        qs = max(128 * t - qbase, 0)
                        sc_ps = ps_sc.tile([128, 1024], f32, tag="sc")
                        for lo, hi in _col_pieces(qs):
                            nc.tensor.matmul(
                                sc_ps[:, lo:hi],
                                ktr[hsl, 128 * t : 128 * (t + 1)],
                                qtr[hsl, qbase + lo : qbase + hi],
                                start=True,
                                stop=True,
                            )
                        pt = ptp.tile([128, 1024], fp16, tag="pt")
                        nc.scalar.activation(
                            pt[:, qs:1024], sc_ps[:, qs:1024], EXP, scale=SCALE
                        )
                        if 128 * t >= qbase:  # diagonal tile: causal mask
                            nc.gpsimd.affine_select(
                                out=pt[:, qs : qs + 128],
                                in_=pt[:, qs : qs + 128],
                                pattern=[[1, 128]],
                                compare_op=IS_GE,
                                fill=0.0,
                                base=0,
                                channel_multiplier=-1,
                            )
                        if pending is not None:
                            emit_pv(*pending)
                        pending = (t, pt, qs)
                    emit_pv(*pending)
                    # epilogue: per 128-wide qr tile
                    for q4 in range(8):
                        qt_i = 8 * cch + q4
                        at_sb = epi.tile([128, 128], fp16, tag="at_sb")
                        nc.any.tensor_copy(
                            at_sb[0:65, :], at_ps[0:65, 128 * q4 : 128 * (q4 + 1)]
                        )
                        tr1 = ps_aux.tile([128, 128], fp16, tag="aux")
                        nc.tensor.transpose(
                            tr1[:, 0:65], at_sb[0:65, :], ident[0:65, 0:65]
                        )
                        den = epi.tile([128, 1], f32, tag="den")
                        nc.vector.reciprocal(den, tr1[:, 64:65])
                        if h == 0:
                            stack = stackp.tile([128, DKH], fp16, tag="stack")
                            stacks[qt_i] = stack
                        nc.vector.tensor_scalar_mul(
                            stacks[qt_i][:, hsl], tr1[:, 0:64], den
                        )
            # ---- output projection ----
            for qt_i in range(NT):
                tr2 = ps_aux.tile([128, 128], fp16, tag="aux")
                nc.tensor.transpose(tr2[:, 0:128], stacks[qt_i], ident)
                ant = epi.tile([128, 128], fp16, tag="ant")
                nc.any.tensor_copy(ant, tr2[:, 0:128])
                osb = outp.tile([128, D], fp16, tag="osb")
                for oc in range(2):
                    po = ps_aux.tile([128, 512], f32, tag="aux")
                    nc.tensor.matmul(
                        po,
                        ant,
                        wo_sb[:, 512 * oc : 512 * (oc + 1)],
                        start=True,
                        stop=True,
                    )
                    nc.any.tensor_copy(osb[:, 512 * oc : 512 * (oc + 1)], po)
                nc.gpsimd.dma_start(out_d[b, 128 * qt_i : 128 * (qt_i + 1), :], osb)

    nc.compile()
    return nc


_NC_CACHE = None


def kernel(x, Wq, Wk, Wv, Wo):
    global _NC_CACHE
    from concourse.bass_utils import run_bass_kernel_spmd

    if _NC_CACHE is None:
        _NC_CACHE = _build_nc()
    nc = _NC_CACHE

    cosT, sinT = _rope_tables()
    p2t = _p2t()
    x16 = np.ascontiguousarray(np.asarray(x, dtype=np.float32).astype(np.float16))
    Wq, Wk, Wv, Wo = (np.asarray(w, dtype=np.float32) for w in (Wq, Wk, Wv, Wo))

    in_maps = []
    for c in range(NCORES):
        rows = slice(c * DKH, (c + 1) * DKH)
        in_maps.append(
            {
                "x": x16,
                "wqt": np.ascontiguousarray(Wq[rows, :].T.astype(np.float16)),
                "wkt": np.ascontiguousarray(Wk[rows, :].T.astype(np.float16)),
                "wvt": np.ascontiguousarray(Wv[rows, :].T.astype(np.float16)),
                "wot": np.ascontiguousarray(Wo[:, rows].T.astype(np.float16)),
                "cosT": cosT,
                "sinT": sinT,
                "p2t": p2t,
            }
        )

    res = run_bass_kernel_spmd(nc, in_maps, core_ids=list(range(NCORES)))
    out = np.zeros((B, S, D), dtype=np.float32)
    for r in res.results:
        out += r["out"].astype(np.float32)
    return out
